# revision 5
# baseline (speedup 1.0000x reference)
"""SeeSaw loss kernel for Trainium2 (8 NeuronCores, batch-parallel).

Math (per batch b, pixel p, with t = target[b,p]):
    M[i,j]     = max(w_i / w_j, 1)
    denom[j,p] = sum_i exp(logit[i,p]) * M[i,j]      (one 128x128 matmul)
    loss_p     = log(denom[t,p]) - logit[t,p]
    out        = mean_p loss_p over all b,h,w

Layout per core (= per batch): classes N=128 on partitions, pixels H*W=16384
along free dim, processed in 8 chunks of 2048.

The matmul runs in bf16 (inputs rounded to bf16; fp32 PSUM accumulation).
Per-element bf16 rounding is random across the 128-term contraction and the
131072-pixel mean, so the final loss keeps ~1e-5 relative accuracy.

Selection of the target row uses DVE scalar_tensor_tensor in bf16 (2x mode):
    (targ_broadcast == iota_per_partition) * X, accumulated per partition
with X = log(denom) (bf16 from ACT) and X = logit (bf16 copy made on GpSimd).
Host combines the 8x[128,16] partial sums (the final mean "all-reduce").
"""

import numpy as np
import ml_dtypes

import concourse.bacc as bacc
import concourse.bass as bass
import concourse.tile as tile
from concourse import mybir

B, N, H, W = 8, 128, 128, 128
HW = H * W
NCHUNK = 8
CW = HW // NCHUNK
F32 = mybir.dt.float32
BF16 = mybir.dt.bfloat16

_NC_CACHE = {}


def _patch_act_tables():
    """Make Exp and Ln resolve to the same activation-table set
    (natural_log_exp_and_others) so the table is loaded once instead of
    thrashing between per-function sets on every chunk."""
    import concourse.bacc as _bacc
    from concourse.hw_specs import get_activation_tables as _orig

    def patched(arch):
        # act_func_set_id is the INDEX into this (ordered) dict, so entries
        # must not be removed or reordered -- only membership is edited.
        tabs = dict(_orig(arch))
        E = mybir.ActivationFunctionType.Exp
        L = mybir.ActivationFunctionType.Ln
        for name in ("exp_and_others", "exp_and_friends", "natural_log"):
            if name in tabs:
                tabs[name] = tabs[name] - {E, L}
        return tabs

    _bacc.get_activation_tables = patched


def _build_nc():
    _patch_act_tables()
    nc = bacc.Bacc("TRN2", target_bir_lowering=False)

    logit = nc.dram_tensor("logit", [N, HW], F32, kind="ExternalInput")
    targ = nc.dram_tensor("targ", [1, HW], BF16, kind="ExternalInput")
    wcol = nc.dram_tensor("wcol", [N, 1], F32, kind="ExternalInput")
    wrow = nc.dram_tensor("wrow", [N, N], F32, kind="ExternalInput")
    iota = nc.dram_tensor("iota", [N, 1], BF16, kind="ExternalInput")
    acc = nc.dram_tensor("acc", [N, 2 * NCHUNK], F32, kind="ExternalOutput")

    with tile.TileContext(nc) as tc:
        with (
            tc.tile_pool(name="singles", bufs=1) as singles,
            tc.tile_pool(name="lg", bufs=3) as lg_pool,
            tc.tile_pool(name="lb", bufs=2) as lb_pool,
            tc.tile_pool(name="ex", bufs=2) as ex_pool,
            tc.tile_pool(name="ld", bufs=2) as ld_pool,
            tc.tile_pool(name="tb", bufs=2) as tb_pool,
            tc.tile_pool(name="scr", bufs=2) as scr_pool,
            tc.tile_pool(name="psum", bufs=2, space="PSUM") as psum_pool,
        ):
            wcol_sb = singles.tile([N, 1], F32)
            wrow_sb = singles.tile([N, N], F32)
            iota_sb = singles.tile([N, 1], BF16)
            nc.sync.dma_start(out=wcol_sb[:], in_=wcol[:])
            nc.sync.dma_start(out=wrow_sb[:], in_=wrow[:])
            nc.sync.dma_start(out=iota_sb[:], in_=iota[:])

            # M[i,j] = max(w_i * (1/w_j), 1);  i = partition, j = free.
            recip = singles.tile([N, N], F32)
            m_raw = singles.tile([N, N], F32)
            m_bf = singles.tile([N, N], BF16)
            nc.vector.reciprocal(out=recip[:], in_=wrow_sb[:])
            nc.vector.tensor_scalar_mul(out=m_raw[:], in0=recip[:], scalar1=wcol_sb[:])
            nc.vector.tensor_scalar_max(out=m_bf[:], in0=m_raw[:], scalar1=1.0)

            acc_sb = singles.tile([N, 2 * NCHUNK], F32)

            for c in range(NCHUNK):
                sl = slice(c * CW, (c + 1) * CW)
                lg_c = lg_pool.tile([N, CW], F32)
                nc.sync.dma_start(out=lg_c[:], in_=logit[:, sl])

                # bf16 logit copy for the numerator-side selection, done
                # on the otherwise-idle GpSimd engine to keep DVE free.
                lb_c = lb_pool.tile([N, CW], BF16)
                nc.gpsimd.tensor_copy(out=lb_c[:], in_=lg_c[:])

                ex_c = ex_pool.tile([N, CW], BF16)
                nc.scalar.activation(
                    out=ex_c[:], in_=lg_c[:], func=mybir.ActivationFunctionType.Exp
                )

                ps_c = psum_pool.tile([N, CW], F32)
                for j in range(CW // 512):
                    jsl = slice(j * 512, (j + 1) * 512)
                    nc.tensor.matmul(
                        ps_c[:, jsl], m_bf[:], ex_c[:, jsl], start=True, stop=True
                    )

                ld_c = ld_pool.tile([N, CW], BF16)
                nc.scalar.activation(
                    out=ld_c[:], in_=ps_c[:], func=mybir.ActivationFunctionType.Ln
                )

                tb_c = tb_pool.tile([N, CW], BF16)
                nc.sync.dma_start(out=tb_c[:], in_=targ[0:1, sl].to_broadcast([N, CW]))

                s1 = scr_pool.tile([N, CW], BF16, tag="scr")
                nc.vector.scalar_tensor_tensor(
                    out=s1[:],
                    in0=tb_c[:],
                    scalar=iota_sb[:],
                    in1=ld_c[:],
                    op0=mybir.AluOpType.is_equal,
                    op1=mybir.AluOpType.mult,
                    accum_out=acc_sb[:, 2 * c : 2 * c + 1],
                )
                s2 = scr_pool.tile([N, CW], BF16, tag="scr")
                nc.vector.scalar_tensor_tensor(
                    out=s2[:],
                    in0=tb_c[:],
                    scalar=iota_sb[:],
                    in1=lb_c[:],
                    op0=mybir.AluOpType.is_equal,
                    op1=mybir.AluOpType.mult,
                    accum_out=acc_sb[:, 2 * c + 1 : 2 * c + 2],
                )

            nc.sync.dma_start(out=acc[:], in_=acc_sb[:])

    nc.compile()
    return nc


def get_nc():
    if "nc" not in _NC_CACHE:
        _NC_CACHE["nc"] = _build_nc()
    return _NC_CACHE["nc"]


def make_in_maps(logit, target, weight):
    """Shard per batch: core b gets batch b."""
    logit = np.ascontiguousarray(np.asarray(logit, dtype=np.float32))
    target = np.asarray(target)
    weight = np.ascontiguousarray(np.asarray(weight, dtype=np.float32))
    targ_bf = target.astype(np.float32).astype(ml_dtypes.bfloat16)
    iota = np.arange(N, dtype=np.float32).astype(ml_dtypes.bfloat16).reshape(N, 1)
    in_maps = []
    for b in range(B):
        in_maps.append(
            {
                "logit": logit[b].reshape(N, HW),
                "targ": targ_bf[b].reshape(1, HW),
                "wcol": weight[b].reshape(N, 1),
                "wrow": np.ascontiguousarray(np.tile(weight[b].reshape(1, N), (N, 1))),
                "iota": iota,
            }
        )
    return in_maps


def combine(results):
    tot = np.float64(0.0)
    for r in results:
        a = r["acc"].astype(np.float64)
        tot += a[:, 0::2].sum() - a[:, 1::2].sum()
    return np.float32(tot / (B * HW))


def kernel(logit, target, weight, epoch=None, **_ignored):
    from concourse.bass_utils import run_bass_kernel_spmd

    nc = get_nc()
    in_maps = make_in_maps(logit, target, weight)
    res = run_bass_kernel_spmd(nc, in_maps, core_ids=list(range(B)))
    return combine(res.results)


# revision 6
# speedup vs baseline: 1.5355x; 1.5355x over previous
"""SeeSaw loss kernel for Trainium2 (8 NeuronCores, batch-parallel).

Math (per batch b, pixel p, with t = target[b,p]):
    M[i,j]     = max(w_i / w_j, 1)
    denom[j,p] = sum_i exp(logit[i,p]) * M[i,j]      (one 128x128 matmul)
    loss_p     = log(denom[t,p]) - logit[t,p]
    out        = mean_p loss_p over all b,h,w

Layout per core (= per batch): classes N=128 on partitions, pixels H*W=16384
along free dim, processed in 8 chunks of 2048.

The matmul runs in bf16 (inputs rounded to bf16; fp32 PSUM accumulation).
Per-element bf16 rounding is random across the 128-term contraction and the
131072-pixel mean, so the final loss keeps ~1e-5 relative accuracy.

Selection of the target row uses DVE scalar_tensor_tensor in bf16 (2x mode):
    (targ_broadcast == iota_per_partition) * X, accumulated per partition
with X = log(denom) (bf16 from ACT) and X = logit (bf16 copy made on GpSimd).
Host combines the 8x[128,16] partial sums (the final mean "all-reduce").
"""

import numpy as np
import ml_dtypes

import concourse.bacc as bacc
import concourse.bass as bass
import concourse.tile as tile
from concourse import mybir

B, N, H, W = 8, 128, 128, 128
HW = H * W
NCHUNK = 8
CW = HW // NCHUNK
F32 = mybir.dt.float32
BF16 = mybir.dt.bfloat16

_NC_CACHE = {}


def _patch_act_tables():
    """Make Exp and Ln resolve to the same activation-table set
    (natural_log_exp_and_others) so the table is loaded once instead of
    thrashing between per-function sets on every chunk."""
    import concourse.bacc as _bacc
    from concourse.hw_specs import get_activation_tables as _orig

    def patched(arch):
        # act_func_set_id is the INDEX into this (ordered) dict, so entries
        # must not be removed or reordered -- only membership is edited.
        tabs = dict(_orig(arch))
        E = mybir.ActivationFunctionType.Exp
        L = mybir.ActivationFunctionType.Ln
        for name in ("exp_and_others", "exp_and_friends", "natural_log"):
            if name in tabs:
                tabs[name] = tabs[name] - {E, L}
        return tabs

    _bacc.get_activation_tables = patched


def _build_nc():
    _patch_act_tables()
    nc = bacc.Bacc("TRN2", target_bir_lowering=False)

    logit = nc.dram_tensor("logit", [N, HW], F32, kind="ExternalInput")
    targ = nc.dram_tensor("targ", [1, HW], BF16, kind="ExternalInput")
    wcol = nc.dram_tensor("wcol", [N, 1], F32, kind="ExternalInput")
    wrow = nc.dram_tensor("wrow", [N, N], F32, kind="ExternalInput")
    iota = nc.dram_tensor("iota", [N, 1], BF16, kind="ExternalInput")
    acc = nc.dram_tensor("acc", [N, 2 * NCHUNK], F32, kind="ExternalOutput")

    with tile.TileContext(nc) as tc:
        with (
            tc.tile_pool(name="singles", bufs=1) as singles,
            tc.tile_pool(name="lg", bufs=3) as lg_pool,
            tc.tile_pool(name="lb", bufs=2) as lb_pool,
            tc.tile_pool(name="ex", bufs=2) as ex_pool,
            tc.tile_pool(name="ld", bufs=2) as ld_pool,
            tc.tile_pool(name="tb", bufs=2) as tb_pool,
            tc.tile_pool(name="scr", bufs=2) as scr_pool,
            tc.tile_pool(name="psum", bufs=2, space="PSUM") as psum_pool,
        ):
            wcol_sb = singles.tile([N, 1], F32)
            wrow_sb = singles.tile([N, N], F32)
            iota_sb = singles.tile([N, 1], BF16)
            nc.sync.dma_start(out=wcol_sb[:], in_=wcol[:])
            nc.sync.dma_start(out=wrow_sb[:], in_=wrow[:])
            nc.sync.dma_start(out=iota_sb[:], in_=iota[:])

            # M[i,j] = max(w_i * (1/w_j), 1);  i = partition, j = free.
            recip = singles.tile([N, N], F32)
            m_raw = singles.tile([N, N], F32)
            m_bf = singles.tile([N, N], BF16)
            nc.vector.reciprocal(out=recip[:], in_=wrow_sb[:])
            nc.vector.tensor_scalar_mul(out=m_raw[:], in0=recip[:], scalar1=wcol_sb[:])
            nc.vector.tensor_scalar_max(out=m_bf[:], in0=m_raw[:], scalar1=1.0)

            acc_sb = singles.tile([N, 2 * NCHUNK], F32)

            for c in range(NCHUNK):
                sl = slice(c * CW, (c + 1) * CW)
                lg_c = lg_pool.tile([N, CW], F32)
                nc.sync.dma_start(out=lg_c[:], in_=logit[:, sl])

                # bf16 logit copy for the numerator-side selection (DVE 2x;
                # GpSimd is ~7x slower here and contends with DVE's SBUF port)
                lb_c = lb_pool.tile([N, CW], BF16)
                nc.vector.tensor_copy(out=lb_c[:], in_=lg_c[:])

                ex_c = ex_pool.tile([N, CW], BF16)
                nc.scalar.activation(
                    out=ex_c[:], in_=lg_c[:], func=mybir.ActivationFunctionType.Exp
                )

                ps_c = psum_pool.tile([N, CW], F32)
                for j in range(CW // 512):
                    jsl = slice(j * 512, (j + 1) * 512)
                    nc.tensor.matmul(
                        ps_c[:, jsl], m_bf[:], ex_c[:, jsl], start=True, stop=True
                    )

                ld_c = ld_pool.tile([N, CW], BF16)
                nc.scalar.activation(
                    out=ld_c[:], in_=ps_c[:], func=mybir.ActivationFunctionType.Ln
                )

                tb_c = tb_pool.tile([N, CW], BF16)
                nc.sync.dma_start(out=tb_c[:], in_=targ[0:1, sl].to_broadcast([N, CW]))

                s1 = scr_pool.tile([N, CW], BF16, tag="scr")
                nc.vector.scalar_tensor_tensor(
                    out=s1[:],
                    in0=tb_c[:],
                    scalar=iota_sb[:],
                    in1=ld_c[:],
                    op0=mybir.AluOpType.is_equal,
                    op1=mybir.AluOpType.mult,
                    accum_out=acc_sb[:, 2 * c : 2 * c + 1],
                )
                s2 = scr_pool.tile([N, CW], BF16, tag="scr")
                nc.vector.scalar_tensor_tensor(
                    out=s2[:],
                    in0=tb_c[:],
                    scalar=iota_sb[:],
                    in1=lb_c[:],
                    op0=mybir.AluOpType.is_equal,
                    op1=mybir.AluOpType.mult,
                    accum_out=acc_sb[:, 2 * c + 1 : 2 * c + 2],
                )

            nc.sync.dma_start(out=acc[:], in_=acc_sb[:])

    nc.compile()
    return nc


def get_nc():
    if "nc" not in _NC_CACHE:
        _NC_CACHE["nc"] = _build_nc()
    return _NC_CACHE["nc"]


def make_in_maps(logit, target, weight):
    """Shard per batch: core b gets batch b."""
    logit = np.ascontiguousarray(np.asarray(logit, dtype=np.float32))
    target = np.asarray(target)
    weight = np.ascontiguousarray(np.asarray(weight, dtype=np.float32))
    targ_bf = target.astype(np.float32).astype(ml_dtypes.bfloat16)
    iota = np.arange(N, dtype=np.float32).astype(ml_dtypes.bfloat16).reshape(N, 1)
    in_maps = []
    for b in range(B):
        in_maps.append(
            {
                "logit": logit[b].reshape(N, HW),
                "targ": targ_bf[b].reshape(1, HW),
                "wcol": weight[b].reshape(N, 1),
                "wrow": np.ascontiguousarray(np.tile(weight[b].reshape(1, N), (N, 1))),
                "iota": iota,
            }
        )
    return in_maps


def combine(results):
    tot = np.float64(0.0)
    for r in results:
        a = r["acc"].astype(np.float64)
        tot += a[:, 0::2].sum() - a[:, 1::2].sum()
    return np.float32(tot / (B * HW))


def kernel(logit, target, weight, epoch=None, **_ignored):
    from concourse.bass_utils import run_bass_kernel_spmd

    nc = get_nc()
    in_maps = make_in_maps(logit, target, weight)
    res = run_bass_kernel_spmd(nc, in_maps, core_ids=list(range(B)))
    return combine(res.results)
